# revision 20
# baseline (speedup 1.0000x reference)
"""Causal self-attention (B=4, T=2048, C=1024, H=16, D=64) on 8 TRN2 NeuronCores.

Sharding: core c handles batch b = c//2 and head-group hg = c%2 (8 of 16 heads).
Per core: column-sharded QKV projection (only its heads' q/k/v columns, only its
batch's rows), full causal attention for its 8 heads, row-sharded output
projection producing a partial [T, C] result. Host sums the two head-group
partials per batch (the "all-reduce") and adds the bias correction term.

Math notes:
 - k-bias is dropped: softmax((q+bq)@(k+bk)^T) == softmax((q+bq)@k^T) because
   the (q+bq)@bk term is constant along the key axis.
 - v-bias and proj-bias are folded into a host-side correction: since softmax
   rows sum to 1, y = P@(V + 1 bv^T) = P@V + 1 bv^T, so the output correction
   is bv @ w_proj + b_proj added to every row.
 - Attention works fully in S^T/y^T layout: S^T = K^T.T @ Q^T (head pairs
   packed into the PE via tile_position), exp on ACT, causal mask multiplied
   into the 128-wide triangle block only, and wide PV y^T[65,512] += V'.T@expS
   with a ones-column in V' producing the softmax denominators in row 64.
   Normalization happens in y^T layout: reciprocal of the sums row, a tiny DMA
   moves it to partition 0, a GpSimd partition_broadcast fans it across
   partitions, and one DVE multiply scales y^T — no PE transposes anywhere.
   The projection then consumes normalized y^T directly as its stationary
   operand. Head 1 of each pair is shifted to partitions 64-127 by a small
   SBUF-to-SBUF DMA (engines are partition-locked; DMA is not).

Schedule: attention for query block q5 is emitted one row-slab behind the QKV
projection (it needs rows <= (q5+1)*512 only), and the output projection for
those rows follows immediately, so the ACT exp stream (the attention
bottleneck) overlaps the PE-heavy projection matmuls throughout.
"""

import numpy as np
import ml_dtypes

B, T, C, H, D = 4, 2048, 1024, 16, 64
HC = 8            # heads per core
KCH = C // 128    # 8 contraction chunks
RC = T // 128     # 16 row chunks
QQ = T // 512     # 4 query super-blocks
BF16 = ml_dtypes.bfloat16

_COMPILED = {}


def _build_nc():
    import concourse.bass as bass
    from concourse import bacc
    import concourse.tile as tile
    from concourse import mybir

    bf16 = mybir.dt.bfloat16
    f32 = mybir.dt.float32
    EXP = mybir.ActivationFunctionType.Exp
    ADD = mybir.AluOpType.add
    MULT = mybir.AluOpType.mult

    nc = bacc.Bacc(None, target_bir_lowering=False)

    xT = nc.dram_tensor("xT", [128, KCH, T], bf16, kind="ExternalInput")
    wqk = nc.dram_tensor("wqk", [128, KCH, 8, 128], bf16, kind="ExternalInput")
    wv = nc.dram_tensor("wv", [128, KCH, 512], bf16, kind="ExternalInput")
    bq = nc.dram_tensor("bq", [128, 4], f32, kind="ExternalInput")
    wp = nc.dram_tensor("wp", [128, 4, 1024], bf16, kind="ExternalInput")
    out = nc.dram_tensor("out", [T, C], f32, kind="ExternalOutput")

    # Causal mask for the diagonal 128-key x 512-q blocks, variant r = kc % 4:
    # valid iff r*128 + k <= q. Applied multiplicatively to exp(S) in bf16.
    kk = np.arange(128)[:, None, None]
    rr = np.arange(4)[None, :, None]
    qq = np.arange(512)[None, None, :]
    mask_np = (rr * 128 + kk <= qq).astype(BF16)
    msk = nc.inline_tensor(mask_np, name="msk")

    with tile.TileContext(nc) as tc:
        with tc.tile_pool(name="singles", bufs=1) as singles:
            wqk_sb = singles.tile([128, KCH, 8, 128], bf16)
            wv_sb = singles.tile([128, KCH, 512], bf16)
            bq_sb = singles.tile([128, 4], f32)
            wp_sb = singles.tile([128, 4, 1024], bf16)
            msk_sb = singles.tile([128, 4, 512], bf16)

            # persistent activations
            qT_sb = singles.tile([128, 4, T], bf16)  # q^T, heads 2c,2c+1 in chunk c
            kT_sb = singles.tile([128, 4, T], bf16)
            v_sb = singles.tile([128, RC, HC, 65], bf16)  # V natural + ones col
            yT_sb = singles.tile([128, 4, T], bf16)       # normalized y^T

            # weights needed first go on the SP ring; the rest on the ACT ring
            # so they don't delay the first projection matmuls
            nc.sync.dma_start(wqk_sb[:], wqk[:])
            nc.sync.dma_start(wv_sb[:], wv[:])
            nc.gpsimd.dma_start(bq_sb[:], bq[:])
            nc.gpsimd.dma_start(msk_sb[:], msk[:])
            nc.gpsimd.dma_start(wp_sb[:], wp[:])
            nc.vector.memset(v_sb[:, :, :, 64], 1.0)

            with tc.tile_pool(name="xt", bufs=2) as xp, \
                 tc.tile_pool(name="att", bufs=3) as ap_, \
                 tc.tile_pool(name="nrm", bufs=4) as np_, \
                 tc.tile_pool(name="outp", bufs=2) as op_, \
                 tc.tile_pool(name="psA", bufs=2, space="PSUM") as psA, \
                 tc.tile_pool(name="psS", bufs=2, space="PSUM") as psS, \
                 tc.tile_pool(name="psYT", bufs=1, space="PSUM") as psYT:

                def qkv(r5):
                    sl = slice(r5 * 512, (r5 + 1) * 512)
                    xt = xp.tile([128, KCH, 512], bf16)
                    nc.sync.dma_start(xt[:], xT[:, :, sl])
                    for cc in range(8):      # qk col chunks (0-3 q, 4-7 k)
                        ps = psA.tile([128, 512], f32, tag="acc", name="psqk")
                        for kc in range(KCH):
                            nc.tensor.matmul(
                                ps[:], wqk_sb[:, kc, cc, :], xt[:, kc, :],
                                start=(kc == 0), stop=(kc == KCH - 1))
                        if cc < 4:
                            nc.vector.tensor_scalar(
                                out=qT_sb[:, cc, sl], in0=ps[:],
                                scalar1=bq_sb[:, cc:cc + 1], scalar2=0.125,
                                op0=ADD, op1=MULT)
                        else:
                            nc.vector.tensor_copy(out=kT_sb[:, cc - 4, sl],
                                                  in_=ps[:])
                    for rs in range(4):      # v rows, 128 at a time
                        rc = r5 * 4 + rs
                        psv = psA.tile([128, 512], f32, tag="acc", name="psv")
                        for kc in range(KCH):
                            nc.tensor.matmul(
                                psv[:], xt[:, kc, rs * 128:(rs + 1) * 128],
                                wv_sb[:, kc, :],
                                start=(kc == 0), stop=(kc == KCH - 1))
                        nc.vector.tensor_copy(
                            out=v_sb[:, rc, :, 0:64],
                            in_=psv[:].rearrange("p (h d) -> p h d", h=HC))

                def attn(q5):
                    nkc = 4 * (q5 + 1)
                    ssl = slice(q5 * 512, (q5 + 1) * 512)
                    for pr in range(4):      # head pair: local heads 2pr, 2pr+1
                        psyt = [psYT.tile([65, 512], f32, name=f"psyt{i}")
                                for i in range(2)]
                        for kc in range(nkc):
                            diag = (kc // 4 == q5)
                            r = kc % 4
                            qof = r * 128 if diag else 0   # causal column trim
                            pss = psS.tile([128, 2, 512], f32)
                            for i in range(2):   # heads packed in the PE
                                po = i * 64
                                nc.tensor.matmul(
                                    pss[:, i, qof:],
                                    kT_sb[po:po + 64, pr,
                                          kc * 128:(kc + 1) * 128],
                                    qT_sb[po:po + 64, pr,
                                          q5 * 512 + qof:(q5 + 1) * 512],
                                    start=True, stop=True)
                            exps = ap_.tile([128, 2, 512], bf16)
                            nc.scalar.activation(exps[:, :, qof:],
                                                 pss[:, :, qof:], EXP)
                            for i in range(2):
                                h = 2 * pr + i
                                if diag:
                                    # only the triangle block needs masking
                                    nc.vector.tensor_mul(
                                        out=exps[:, i, qof:qof + 128],
                                        in0=exps[:, i, qof:qof + 128],
                                        in1=msk_sb[:, r, qof:qof + 128])
                                nc.tensor.matmul(
                                    psyt[i][:, qof:], v_sb[:, kc, h, :],
                                    exps[:, i, qof:],
                                    start=(kc == 0), stop=(kc == nkc - 1))
                        for i in range(2):
                            # normalize in y^T layout, no transposes:
                            # recip(sums row) -> shift to partition 0 ->
                            # broadcast down partitions -> multiply
                            ytf = ap_.tile([65, 512], f32, name="ytf")
                            nc.vector.tensor_copy(out=ytf[:], in_=psyt[i][:])
                            nc.vector.reciprocal(ytf[64:65, :], ytf[64:65, :])
                            rsrc = np_.tile([1, 512], f32, name="rsrc")
                            nc.gpsimd.dma_start(rsrc[:], ytf[64:65, :])
                            rb = np_.tile([64, 512], f32, name="rb")
                            nc.gpsimd.partition_broadcast(rb[:], rsrc[0:1, :])
                            if i == 0:
                                nc.vector.tensor_mul(
                                    out=yT_sb[0:64, pr, ssl],
                                    in0=ytf[0:64, :], in1=rb[:])
                            else:
                                tmp = np_.tile([64, 512], bf16, name="tmp")
                                nc.vector.tensor_mul(
                                    out=tmp[:], in0=ytf[0:64, :], in1=rb[:])
                                # partition shift 0-63 -> 64-127 (DMA only)
                                nc.gpsimd.dma_start(yT_sb[64:128, pr, ssl],
                                                    tmp[:])

                def proj(q5):
                    for rs in range(4):
                        rc = q5 * 4 + rs
                        pso = [psA.tile([128, 512], f32, tag="acc",
                                        name=f"pso{oh}") for oh in range(2)]
                        for t in range(4):
                            lhs = yT_sb[:, t, rc * 128:(rc + 1) * 128]
                            for oh in range(2):
                                nc.tensor.matmul(
                                    pso[oh][:], lhs,
                                    wp_sb[:, t, oh * 512:(oh + 1) * 512],
                                    start=(t == 0), stop=(t == 3))
                        osb = op_.tile([128, 1024], f32)
                        for oh in range(2):
                            nc.vector.tensor_copy(
                                out=osb[:, oh * 512:(oh + 1) * 512],
                                in_=pso[oh][:])
                        nc.gpsimd.dma_start(out[rc * 128:(rc + 1) * 128, :],
                                            osb[:])

                for r5 in range(4):
                    qkv(r5)
                    if r5 >= 1:
                        attn(r5 - 1)
                        proj(r5 - 1)
                attn(3)
                proj(3)

    nc.compile()
    return nc


def _prep_core_inputs(x, w_attn, b_attn, w_proj, c):
    b, hg = c // 2, c % 2
    xb = np.ascontiguousarray(x[b])                       # [T, C]
    xT = np.ascontiguousarray(
        xb.T.reshape(KCH, 128, T).transpose(1, 0, 2)).astype(BF16)
    wq = w_attn[:, hg * 512:(hg + 1) * 512]
    wk = w_attn[:, C + hg * 512:C + (hg + 1) * 512]
    wqk = np.concatenate([wq, wk], axis=1)                # [C, 1024]
    wqk = np.ascontiguousarray(
        wqk.reshape(KCH, 128, 8, 128).transpose(1, 0, 2, 3)).astype(BF16)
    wv = w_attn[:, 2 * C + hg * 512:2 * C + (hg + 1) * 512]
    wv = np.ascontiguousarray(
        wv.reshape(KCH, 128, 512).transpose(1, 0, 2)).astype(BF16)
    bqv = np.ascontiguousarray(
        b_attn[hg * 512:(hg + 1) * 512].reshape(4, 128).T).astype(np.float32)
    wpc = w_proj[hg * 512:(hg + 1) * 512, :]
    wpc = np.ascontiguousarray(
        wpc.reshape(4, 128, 1024).transpose(1, 0, 2)).astype(BF16)
    return {"xT": xT, "wqk": wqk, "wv": wv, "bq": bqv, "wp": wpc}


def _run(nc, in_maps, **kwargs):
    from concourse.bass_utils import run_bass_kernel_spmd
    return run_bass_kernel_spmd(nc, in_maps, core_ids=list(range(8)), **kwargs)


def kernel(x, w_attn, b_attn, w_proj, b_proj, _trace=False):
    x = np.asarray(x, dtype=np.float32)
    w_attn = np.asarray(w_attn, dtype=np.float32)
    b_attn = np.asarray(b_attn, dtype=np.float32)
    w_proj = np.asarray(w_proj, dtype=np.float32)
    b_proj = np.asarray(b_proj, dtype=np.float32)

    if "nc" not in _COMPILED:
        _COMPILED["nc"] = _build_nc()
    nc = _COMPILED["nc"]

    in_maps = [_prep_core_inputs(x, w_attn, b_attn, w_proj, c) for c in range(8)]
    kwargs = {"trace": True} if _trace else {}
    res = _run(nc, in_maps, **kwargs)
    _COMPILED["last_result"] = res

    corr = b_attn[2 * C:].astype(np.float32) @ w_proj + b_proj
    out = np.empty((B, T, C), np.float32)
    for b in range(B):
        out[b] = res.results[2 * b]["out"] + res.results[2 * b + 1]["out"]
        out[b] += corr[None, :]
    return out


# revision 24
# speedup vs baseline: 1.3269x; 1.3269x over previous
"""Causal self-attention (B=4, T=2048, C=1024, H=16, D=64) on 8 TRN2 NeuronCores.

Sharding: core c handles batch b = c//2 and head-group hg = c%2 (8 of 16 heads).
Per core: column-sharded QKV projection (only its heads' q/k/v columns, only its
batch's rows), full causal attention for its 8 heads, row-sharded output
projection producing a partial [T, C] result. Host sums the two head-group
partials per batch (the "all-reduce") and adds the bias correction term.

Math notes:
 - k-bias is dropped: softmax((q+bq)@(k+bk)^T) == softmax((q+bq)@k^T) because
   the (q+bq)@bk term is constant along the key axis.
 - v-bias and proj-bias are folded into a host-side correction: since softmax
   rows sum to 1, y = P@(V + 1 bv^T) = P@V + 1 bv^T, so the output correction
   is bv @ w_proj + b_proj added to every row.
 - Attention works in S^T layout ([keys, q]): softmax denominators come from a
   ones-column appended to V (row 64 of the PV accumulation), and the PV
   matmul P^T.T @ V' = P @ V' lands y in natural [q, d] layout so the
   normalization is a per-partition scalar multiply.
"""

import numpy as np
import ml_dtypes

B, T, C, H, D = 4, 2048, 1024, 16, 64
HC = 8            # heads per core
KCH = C // 128    # 8 contraction chunks
RC = T // 128     # 16 row chunks
QQ = T // 512     # 4 query super-blocks
BF16 = ml_dtypes.bfloat16

_COMPILED = {}


def _build_nc():
    from concourse import bacc
    import concourse.tile as tile
    from concourse import mybir

    bf16 = mybir.dt.bfloat16
    f32 = mybir.dt.float32
    EXP = mybir.ActivationFunctionType.Exp
    ADD = mybir.AluOpType.add
    MULT = mybir.AluOpType.mult

    nc = bacc.Bacc(None, target_bir_lowering=False)

    xT = nc.dram_tensor("xT", [128, KCH, T], bf16, kind="ExternalInput")
    wqk = nc.dram_tensor("wqk", [128, KCH, 8, 128], bf16, kind="ExternalInput")
    wv = nc.dram_tensor("wv", [128, KCH, 512], bf16, kind="ExternalInput")
    bq = nc.dram_tensor("bq", [128, 4], f32, kind="ExternalInput")
    wp = nc.dram_tensor("wp", [128, 4, 1024], bf16, kind="ExternalInput")
    out = nc.dram_tensor("out", [T, C], f32, kind="ExternalOutput")

    # Causal mask for the diagonal 128-key x 512-q blocks, variant r = kc % 4:
    # valid iff r*128 + k <= q. Applied multiplicatively to exp(S) in bf16.
    kk = np.arange(128)[:, None, None]
    rr = np.arange(4)[None, :, None]
    qq = np.arange(512)[None, None, :]
    mask_np = (rr * 128 + kk <= qq).astype(BF16)
    msk = nc.inline_tensor(mask_np, name="msk")

    with tile.TileContext(nc) as tc:
        with tc.tile_pool(name="singles", bufs=1) as singles:
            wqk_sb = singles.tile([128, KCH, 8, 128], bf16)
            wv_sb = singles.tile([128, KCH, 512], bf16)
            bq_sb = singles.tile([128, 4], f32)
            wp_sb = singles.tile([128, 4, 1024], bf16)
            msk_sb = singles.tile([128, 4, 512], bf16)
            # weights needed first go on the SP ring; the rest on the ACT ring
            # so they don't delay the first projection matmuls
            nc.sync.dma_start(wqk_sb[:], wqk[:])
            nc.sync.dma_start(wv_sb[:], wv[:])
            nc.scalar.dma_start(bq_sb[:], bq[:])
            nc.scalar.dma_start(msk_sb[:], msk[:])
            nc.scalar.dma_start(wp_sb[:], wp[:])

            # persistent activations
            qT_sb = singles.tile([128, 4, T], bf16)   # q^T, heads 2c,2c+1 in chunk c
            kT_sb = singles.tile([128, 4, T], bf16)
            v_sb = singles.tile([128, RC, HC, 65], bf16)  # V natural + ones col
            yT_sb = singles.tile([128, 4, T], bf16)       # normalized y^T

            nc.vector.memset(v_sb[:, :, :, 64], 1.0)

            # ---- Phase 1: QKV projection ----
            with tc.tile_pool(name="xt", bufs=3) as xp, \
                 tc.tile_pool(name="psA", bufs=4, space="PSUM") as psA:
                for r5 in range(4):           # 512-row chunks
                    sl = slice(r5 * 512, (r5 + 1) * 512)
                    xt = xp.tile([128, KCH, 512], bf16)
                    nc.sync.dma_start(xt[:], xT[:, :, sl])
                    for cc in range(8):       # qk column chunks (0-3 q, 4-7 k)
                        ps = psA.tile([128, 512], f32)
                        for kc in range(KCH):
                            nc.tensor.matmul(ps[:], wqk_sb[:, kc, cc, :],
                                             xt[:, kc, :],
                                             start=(kc == 0), stop=(kc == KCH - 1))
                        if cc < 4:
                            nc.vector.tensor_scalar(
                                out=qT_sb[:, cc, sl], in0=ps[:],
                                scalar1=bq_sb[:, cc:cc + 1], scalar2=0.125,
                                op0=ADD, op1=MULT)
                        else:
                            nc.vector.tensor_copy(out=kT_sb[:, cc - 4, sl], in_=ps[:])
                    for rs in range(4):       # v rows, 128 at a time
                        rc = r5 * 4 + rs
                        psv = psA.tile([128, 512], f32)
                        for kc in range(KCH):
                            nc.tensor.matmul(psv[:], xt[:, kc, rs * 128:(rs + 1) * 128],
                                             wv_sb[:, kc, :],
                                             start=(kc == 0), stop=(kc == KCH - 1))
                        nc.vector.tensor_copy(
                            out=v_sb[:, rc, :, 0:64],
                            in_=psv[:].rearrange("p (h d) -> p h d", h=HC))

            # ---- Phase 2: attention (head pairs packed via tile_position) ----
            # S^T = K^T.T @ Q^T per 128-key chunk (two heads packed in the PE
            # array); exp on ACT; causal mask multiplied into exp(S) in bf16;
            # wide PV: y^T[65,512] += V'.T @ expS with V' stationary.
            # Normalization stays in y^T layout (no PE transposes): recip of
            # the sums row, tiny DMA shifts it to partition 0, GpSimd
            # partition_broadcast fans it down, one DVE multiply scales y^T.
            # Head 1 of each pair is DMA-shifted to partitions 64-127.
            with tc.tile_pool(name="att", bufs=3) as ap, \
                 tc.tile_pool(name="nrm", bufs=4) as np_, \
                 tc.tile_pool(name="psS", bufs=3, space="PSUM") as psS, \
                 tc.tile_pool(name="psYT", bufs=1, space="PSUM") as psYT:
                for pr in range(4):           # head pair: local heads 2pr, 2pr+1
                    for q5 in range(QQ):
                        qsl = slice(q5 * 512, (q5 + 1) * 512)
                        psyt = [psYT.tile([65, 512], f32, name=f"psyt{i}")
                                for i in range(2)]
                        nkc = 4 * (q5 + 1)
                        for kc in range(nkc):
                            diag = (kc // 4 == q5)
                            r = kc % 4
                            qof = r * 128 if diag else 0  # causal column trim
                            pss = psS.tile([128, 2, 512], f32)
                            for i in range(2):   # head in pair, packed in PE
                                po = i * 64
                                nc.tensor.matmul(
                                    pss[:, i, qof:],
                                    kT_sb[po:po + 64, pr, kc * 128:(kc + 1) * 128],
                                    qT_sb[po:po + 64, pr,
                                          q5 * 512 + qof:(q5 + 1) * 512],
                                    start=True, stop=True)
                            exps = ap.tile([128, 2, 512], bf16)
                            nc.scalar.activation(exps[:, :, qof:],
                                                 pss[:, :, qof:], EXP)
                            for i in range(2):
                                h = 2 * pr + i
                                if diag:
                                    # only the 128-col triangle needs masking;
                                    # columns past it are fully valid
                                    nc.vector.tensor_mul(
                                        out=exps[:, i, qof:qof + 128],
                                        in0=exps[:, i, qof:qof + 128],
                                        in1=msk_sb[:, r, qof:qof + 128])
                                nc.tensor.matmul(
                                    psyt[i][:, qof:], v_sb[:, kc, h, :],
                                    exps[:, i, qof:],
                                    start=(kc == 0), stop=(kc == nkc - 1))
                        for i in range(2):
                            ytf = ap.tile([65, 512], f32, name="ytf")
                            nc.vector.tensor_copy(out=ytf[:], in_=psyt[i][:])
                            nc.vector.reciprocal(ytf[64:65, :], ytf[64:65, :])
                            rsrc = np_.tile([1, 512], f32, name="rsrc")
                            nc.sync.dma_start(rsrc[:], ytf[64:65, :])
                            rb = np_.tile([64, 512], f32, name="rb")
                            nc.gpsimd.partition_broadcast(rb[:], rsrc[0:1, :])
                            if i == 0:
                                nc.vector.tensor_mul(
                                    out=yT_sb[0:64, pr, qsl],
                                    in0=ytf[0:64, :], in1=rb[:])
                            else:
                                tmp = np_.tile([64, 512], bf16, name="tmp")
                                nc.vector.tensor_mul(
                                    out=tmp[:], in0=ytf[0:64, :], in1=rb[:])
                                # partition shift 0-63 -> 64-127 (DMA only)
                                nc.sync.dma_start(yT_sb[64:128, pr, qsl],
                                                  tmp[:])

            # ---- Phase 3: output projection (consumes y^T directly) ----
            with tc.tile_pool(name="outp", bufs=3) as op_, \
                 tc.tile_pool(name="psO", bufs=2, space="PSUM") as psO:
                for rc in range(RC):
                    pso = psO.tile([128, 2, 512], f32)
                    for t in range(4):
                        lhs = yT_sb[:, t, rc * 128:(rc + 1) * 128]
                        for oh in range(2):
                            nc.tensor.matmul(pso[:, oh, :], lhs,
                                             wp_sb[:, t, oh * 512:(oh + 1) * 512],
                                             start=(t == 0), stop=(t == 3),
                                             skip_group_check=True)
                    osb = op_.tile([128, 1024], f32)
                    nc.vector.tensor_copy(
                        out=osb[:].rearrange("p (a b) -> p a b", a=2), in_=pso[:])
                    nc.scalar.dma_start(out[rc * 128:(rc + 1) * 128, :], osb[:])

    nc.compile()
    return nc


def _prep_core_inputs(x, w_attn, b_attn, w_proj, c):
    b, hg = c // 2, c % 2
    xb = np.ascontiguousarray(x[b])                       # [T, C]
    xT = np.ascontiguousarray(
        xb.T.reshape(KCH, 128, T).transpose(1, 0, 2)).astype(BF16)
    wq = w_attn[:, hg * 512:(hg + 1) * 512]
    wk = w_attn[:, C + hg * 512:C + (hg + 1) * 512]
    wqk = np.concatenate([wq, wk], axis=1)                # [C, 1024]
    wqk = np.ascontiguousarray(
        wqk.reshape(KCH, 128, 8, 128).transpose(1, 0, 2, 3)).astype(BF16)
    wv = w_attn[:, 2 * C + hg * 512:2 * C + (hg + 1) * 512]
    wv = np.ascontiguousarray(
        wv.reshape(KCH, 128, 512).transpose(1, 0, 2)).astype(BF16)
    bqv = np.ascontiguousarray(
        b_attn[hg * 512:(hg + 1) * 512].reshape(4, 128).T).astype(np.float32)
    wpc = w_proj[hg * 512:(hg + 1) * 512, :]
    wpc = np.ascontiguousarray(
        wpc.reshape(4, 128, 1024).transpose(1, 0, 2)).astype(BF16)
    return {"xT": xT, "wqk": wqk, "wv": wv, "bq": bqv, "wp": wpc}


def _run(nc, in_maps, **kwargs):
    from concourse.bass_utils import run_bass_kernel_spmd
    return run_bass_kernel_spmd(nc, in_maps, core_ids=list(range(8)), **kwargs)


def kernel(x, w_attn, b_attn, w_proj, b_proj, _trace=False):
    x = np.asarray(x, dtype=np.float32)
    w_attn = np.asarray(w_attn, dtype=np.float32)
    b_attn = np.asarray(b_attn, dtype=np.float32)
    w_proj = np.asarray(w_proj, dtype=np.float32)
    b_proj = np.asarray(b_proj, dtype=np.float32)

    if "nc" not in _COMPILED:
        _COMPILED["nc"] = _build_nc()
    nc = _COMPILED["nc"]

    in_maps = [_prep_core_inputs(x, w_attn, b_attn, w_proj, c) for c in range(8)]
    kwargs = {"trace": True} if _trace else {}
    res = _run(nc, in_maps, **kwargs)
    _COMPILED["last_result"] = res

    corr = b_attn[2 * C:].astype(np.float32) @ w_proj + b_proj
    out = np.empty((B, T, C), np.float32)
    for b in range(B):
        out[b] = res.results[2 * b]["out"] + res.results[2 * b + 1]["out"]
        out[b] += corr[None, :]
    return out
